# revision 48
# baseline (speedup 1.0000x reference)
"""Trainium2 Bass kernel for GQA attention with RoPE (B=2, S=1024, HID=2048,
16 q heads / 4 kv heads, head dim 128, causal).

Sharding: 8 cores = 2 batches x 4 kv-head groups. Core c = b*4 + g handles
batch b and kv head g (query heads 4g..4g+3). Each core computes a partial
output y_part = attn_heads @ wo_shard; the host sums the 4 partials per batch.

v12 (final): all-f16 dataflow. Host pre-transposes x and packs every input
into SBUF layout so each DMA is a contiguous per-partition run (no on-device
x transposes; wk/wv previously moved as 256B strided runs, slowing the
fill). Softmax denominator matmuls are halved by pair-summing exp chunks on
the DVE inside each dnpv step, with the PV matmuls emitted first so the PE
covers the add latency. fp8 (incl. DoubleRow) was evaluated and rejected:
HW DoubleRow measures 1.0 cyc/row (2x FLOPs, not the cost model's 4x), and
e4m3's ~6% worst-case element error exceeds the 2e-2 max-error tolerance
through the softmax on every quantization point tested.
"""

import sys

import numpy as np

for _p in ("/opt/trn_rl_repo", "/root/.axon_site/_ro/trn_rl_repo"):
    if _p not in sys.path:
        sys.path.append(_p)

from contextlib import ExitStack

import concourse.bass as bass
import concourse.mybir as mybir
from concourse import bacc
from concourse.masks import make_identity
from concourse.tile import TileContext

P = 128           # partitions / head dim / seq chunk
S = 1024          # sequence length
HID = 2048        # model dim
NH = 4            # query heads per core
D = 128           # head dim
TQ = 256          # query macro-tile
NT = S // TQ      # 4 macro tiles
KC = HID // P     # 16 contraction chunks
NSK = S // P      # 8 key chunks
NG = S // P       # 8 row chunks
F16 = mybir.dt.float16
F32 = mybir.dt.float32
SCALE = 1.0 / float(np.sqrt(D))
AL = mybir.AluOpType
AF = mybir.ActivationFunctionType

N_CORES = 8
B = 2
N_KV = 4
H2 = D // 2


def build_nc(dbg=False):
    nc = bacc.Bacc("TRN2", target_bir_lowering=False, debug=False)
    # all inputs pre-packed by the host into SBUF layout (partition-major,
    # contiguous per-partition runs)
    xt_d = nc.declare_dram_parameter("xt", [P, NG * KC * P], F16, isOutput=False)
    cs_d = nc.declare_dram_parameter("cs", [P, 2 * NG * D], F16, isOutput=False)
    wq_d = nc.declare_dram_parameter("wq", [P, KC * NH * D], F16, isOutput=False)
    wkv_d = nc.declare_dram_parameter("wkv", [P, KC * 2 * D], F16, isOutput=False)
    wo_d = nc.declare_dram_parameter("wo", [P, NH * HID], F16, isOutput=False)
    out_d = nc.declare_dram_parameter("out", [S, HID], F16, isOutput=True)

    with TileContext(nc) as tc, ExitStack() as ctx:
        consts = ctx.enter_context(tc.tile_pool(name="consts", bufs=1))
        wpool = ctx.enter_context(tc.tile_pool(name="wpool", bufs=1))
        persist = ctx.enter_context(tc.tile_pool(name="persist", bufs=1))

        # ---- constants ----
        ident_f32 = consts.tile([P, P], F32, tag="ident_f32")
        make_identity(nc, ident_f32)
        ident = consts.tile([P, P], F16, tag="ident")
        nc.vector.tensor_copy(ident, ident_f32)
        ones = consts.tile([P, P], F16, tag="ones")
        nc.vector.memset(ones, 1.0)
        warm16 = consts.tile([P, 512], F16, tag="warm16")
        nc.vector.memset(warm16, 1.0)
        # causal 0/1 triangle: tri01[k, h, q] = (q >= k), f16, shared by both
        # diagonal chunks of every macro tile
        tri01 = consts.tile([P, 2, P], F16, tag="tri01")
        nc.gpsimd.memset(tri01, 1.0)
        nc.gpsimd.affine_select(
            out=tri01, in_=tri01, compare_op=AL.is_ge, fill=0.0,
            base=0, pattern=[[0, 2], [1, P]], channel_multiplier=-1,
        )

        # ---- persistent weights / tables / activations ----
        # wq in 4 quarter tiles: DMA gating is tile-granular, so quarters
        # let the first q matmuls start after 1MB of DMA instead of 2.5MB
        wq_sbq = [
            wpool.tile([P, 4, NH * D], F16, tag=f"wq{i}", name=f"wq_sbq{i}")
            for i in range(4)
        ]
        wkv_sb = wpool.tile([P, KC, 2 * D], F16, tag="wkv")
        wo_sb = wpool.tile([P, NH, HID], F16, tag="wo")
        cos_sb = wpool.tile([P, NG, D], F16, tag="cos")
        sin_sb = wpool.tile([P, NG, D], F16, tag="sin")

        qT_all = persist.tile([P, NH, S], F16, tag="qT")    # [d, h, sq]
        kT = persist.tile([P, S], F16, tag="kT")            # [d, sk]
        vv = persist.tile([P, NSK, D], F16, tag="vv")       # v natural [sk, d]

        # ---- pools ----
        pa = ctx.enter_context(tc.tile_pool(name="pa", bufs=2))
        pb = ctx.enter_context(tc.tile_pool(name="pb", bufs=2))
        ps_mega = ctx.enter_context(tc.tile_pool(name="ps_mega", bufs=7, space="PSUM"))
        ps_qkv = ctx.enter_context(tc.tile_pool(name="ps_qkv", bufs=1, space="PSUM"))

        # warm the PE clock gate while initial DMAs land
        warm_ps = ps_mega.tile([P, 512], F32, tag="mega", name="warm")
        for _ in range(8):
            nc.tensor.matmul(warm_ps[:, 0:P], ones, ones, start=True, stop=True)
        for _ in range(12):
            nc.tensor.matmul(warm_ps, ones, warm16, start=True, stop=True)
        warm_drain = pa.tile([P, 4], F32, tag="warmdrain", bufs=1)
        nc.vector.tensor_copy(warm_drain, warm_ps[:, 0:4])

        # ---- DMAs: xt per-chunk (sync queue), cos/sin on the scalar queue,
        # weights on the sync queue -- v4 order, packed layouts. ----
        x_tiles = [None] * NG

        def emit_xdma(g):
            xTg = pa.tile([P, KC, P], F16, tag="xT", bufs=4)
            nc.sync.dma_start(
                out=xTg.rearrange("p c d -> p (c d)"),
                in_=xt_d[:, g * KC * P : (g + 1) * KC * P],
            )
            x_tiles[g] = xTg

        emit_xdma(0)
        nc.scalar.dma_start(
            out=cos_sb.rearrange("p g d -> p (g d)"), in_=cs_d[:, 0 : NG * D]
        )
        nc.scalar.dma_start(
            out=sin_sb.rearrange("p g d -> p (g d)"),
            in_=cs_d[:, NG * D : 2 * NG * D],
        )
        for i in range(4):
            nc.sync.dma_start(
                out=wq_sbq[i].rearrange("p c n -> p (c n)"),
                in_=wq_d[:, i * 4 * NH * D : (i + 1) * 4 * NH * D],
            )
        nc.sync.dma_start(
            out=wkv_sb.rearrange("p c d -> p (c d)"), in_=wkv_d[:]
        )
        emit_xdma(1)
        emit_xdma(2)
        emit_xdma(3)
        wo_next = [0]

        def emit_wo_dma():
            h = wo_next[0]
            if h < NH:
                nc.sync.dma_start(
                    out=wo_sb[:, h, :], in_=wo_d[:, h * HID : (h + 1) * HID]
                )
                wo_next[0] += 1

        def bcast_h(ap2d, n):
            """[P, w] slice -> [P, n, w] broadcast AP (0-stride head dim)."""
            return ap2d.rearrange("p (o w) -> p o w", o=1).to_broadcast(
                [P, n, ap2d.shape[-1]]
            )

        # ================= phase A stages =================
        def proj(g):
            """q and kv projections for chunk g (PE, accumulating).
            q uses the dedicated 1-bank pool; kv borrows a mega slot so the
            attention phase gets a 7-deep mega rotation. For g=0 kv goes
            first: it only needs wkv+xt0 (1MB of DMA)."""
            xTg = x_tiles[g]
            q_ps = ps_qkv.tile([P, NH * D], F32, tag="qkv")
            kv_ps = ps_mega.tile([P, 512], F32, tag="mega", name="kv")[:, 0 : 2 * D]
            for c in range(KC):
                nc.tensor.matmul(
                    q_ps, xTg[:, c, :], wq_sbq[c // 4][:, c % 4, :],
                    start=(c == 0), stop=(c == KC - 1),
                )
            for c in range(KC):
                nc.tensor.matmul(
                    kv_ps, xTg[:, c, :], wkv_sb[:, c, :],
                    start=(c == 0), stop=(c == KC - 1),
                )
            qkv_sb = pa.tile([P, NH * D + 2 * D], F16, tag="qkvsb")
            nc.scalar.activation(out=qkv_sb[:, 0 : NH * D], in_=q_ps, func=AF.Copy)
            nc.scalar.activation(
                out=qkv_sb[:, NH * D : NH * D + 2 * D], in_=kv_ps, func=AF.Copy
            )
            return qkv_sb

        def rope_stage(g, qkv_sb):
            """RoPE on q heads (one 4-head strided pass) + k; v copy-out."""
            q3 = qkv_sb[:, 0 : NH * D].rearrange("p (h d) -> p h d", h=NH)
            k2 = qkv_sb[:, NH * D : NH * D + D]
            cos_g = cos_sb[:, g, :]
            sin_g = sin_sb[:, g, :]

            q_rope = pa.tile([P, NH, D], F16, tag="qrope")
            tmpq = pa.tile([P, NH, D], F16, tag="tmpq")
            nc.vector.scalar_tensor_tensor(
                out=tmpq[:, :, 0:H2], in0=q3[:, :, H2:D], scalar=-1.0,
                in1=bcast_h(sin_g[:, 0:H2], NH), op0=AL.mult, op1=AL.mult,
            )
            nc.vector.tensor_tensor(
                out=tmpq[:, :, H2:D], in0=q3[:, :, 0:H2],
                in1=bcast_h(sin_g[:, H2:D], NH), op=AL.mult,
            )
            nc.vector.tensor_tensor(
                out=q_rope, in0=q3, in1=bcast_h(cos_g, NH), op=AL.mult
            )
            nc.vector.tensor_tensor(out=q_rope, in0=q_rope, in1=tmpq, op=AL.add)

            k_rope = pa.tile([P, D], F16, tag="krope")
            tmpk = pa.tile([P, D], F16, tag="tmpk")
            nc.vector.scalar_tensor_tensor(
                out=tmpk[:, 0:H2], in0=k2[:, H2:D], scalar=-1.0,
                in1=sin_g[:, 0:H2], op0=AL.mult, op1=AL.mult,
            )
            nc.vector.tensor_tensor(
                out=tmpk[:, H2:D], in0=k2[:, 0:H2], in1=sin_g[:, H2:D], op=AL.mult
            )
            nc.vector.tensor_tensor(out=k_rope, in0=k2, in1=cos_g, op=AL.mult)
            nc.vector.tensor_tensor(out=k_rope, in0=k_rope, in1=tmpk, op=AL.add)

            nc.vector.tensor_copy(
                vv[:, g, :], qkv_sb[:, NH * D + D : NH * D + 2 * D]
            )
            return q_rope, k_rope

        def rope_transpose(g, q_rope, k_rope):
            """Transpose RoPE'd q/k into persistent qT_all / kT (PE, f16)."""
            tq_ps = ps_mega.tile([P, 4 * P], F16, tag="mega", name="tq")
            for h in range(NH):
                nc.tensor.transpose(
                    tq_ps[:, h * P : (h + 1) * P], q_rope[:, h, :], ident
                )
            nc.vector.tensor_copy(
                qT_all[:, :, g * P : (g + 1) * P],
                tq_ps.rearrange("p (h d) -> p h d", h=NH),
            )
            tk_ps = ps_mega.tile([P, 4 * P], F16, tag="mega", name="tk")
            nc.tensor.transpose(tk_ps[:, 0:P], k_rope, ident)
            nc.vector.tensor_copy(kT[:, g * P : (g + 1) * P], tk_ps[:, 0:P])

        # ================= phase B stages =================
        def scores_step(t, hp):
            """scoresT + exp for head-pair hp of macro tile t -> expst.

            expst[sk, ik, h2, q]: per key chunk ik, both heads of the pair.
            Diagonal chunks get a post-exp 0/1 triangle multiply; the odd
            diagonal chunk only computes the upper query half."""
            q0 = t * TQ
            nsk = 2 * (t + 1)
            expst = pb.tile([P, NSK, 2, TQ], F16, tag="expst", bufs=2)
            for ik in range(nsk):
                s_ps = ps_mega.tile([P, 2 * TQ], F32, tag="mega", name="s")
                s3 = s_ps.rearrange("p (h q) -> p h q", h=2)
                if ik == nsk - 1:  # odd diagonal: queries q0+128..q0+255 only
                    nc.gpsimd.memset(expst[:, ik, :, 0:P], 0.0)
                    nc.tensor.matmul(
                        s3[:, :, P:TQ],
                        kT[:, ik * P : (ik + 1) * P],
                        qT_all[:, 2 * hp : 2 * hp + 2, q0 + P : q0 + TQ],
                        start=True, stop=True,
                    )
                    nc.scalar.activation(
                        out=expst[:, ik, :, P:TQ], in_=s3[:, :, P:TQ],
                        func=AF.Exp, scale=SCALE,
                    )
                    nc.vector.tensor_tensor(
                        out=expst[:, ik, :, P:TQ], in0=expst[:, ik, :, P:TQ],
                        in1=tri01, op=AL.mult,
                    )
                elif t == NT - 1 and hp == 0:
                    # split into q-chunk halves: the first half only needs
                    # ropeT(2t), so the PE isn't gated on the last ropeT
                    nc.tensor.matmul(
                        s3[:, :, 0:P],
                        kT[:, ik * P : (ik + 1) * P],
                        qT_all[:, 0:2, q0 : q0 + P],
                        start=True, stop=True, skip_group_check=True,
                    )
                    nc.tensor.matmul(
                        s3[:, :, P:TQ],
                        kT[:, ik * P : (ik + 1) * P],
                        qT_all[:, 0:2, q0 + P : q0 + TQ],
                        start=True, stop=True, skip_group_check=True,
                    )
                    nc.scalar.activation(
                        out=expst[:, ik, :, :], in_=s3, func=AF.Exp, scale=SCALE,
                    )
                    if ik == nsk - 2:  # even diagonal: lower-left triangle
                        nc.vector.tensor_tensor(
                            out=expst[:, ik, :, 0:P], in0=expst[:, ik, :, 0:P],
                            in1=tri01, op=AL.mult,
                        )
                else:
                    nc.tensor.matmul(
                        s3,
                        kT[:, ik * P : (ik + 1) * P],
                        qT_all[:, 2 * hp : 2 * hp + 2, q0 : q0 + TQ],
                        start=True, stop=True,
                    )
                    nc.scalar.activation(
                        out=expst[:, ik, :, :], in_=s3, func=AF.Exp, scale=SCALE,
                    )
                    if ik == nsk - 2:  # even diagonal: lower-left triangle
                        nc.vector.tensor_tensor(
                            out=expst[:, ik, :, 0:P], in0=expst[:, ik, :, 0:P],
                            in1=tri01, op=AL.mult,
                        )
            return expst

        def dnpv_step(t, hp, expst, uT_t):
            """DVE pair-sums + PV matmuls + halved denominator matmuls,
            then normalize into uT_t. PV runs while the DVE adds complete."""
            nsk = 2 * (t + 1)
            esum = pb.tile([P, NSK // 2, 2, TQ], F16, tag="esum", bufs=2)
            for j in range(nsk // 2):
                nc.vector.tensor_tensor(
                    out=esum[:, j], in0=expst[:, 2 * j], in1=expst[:, 2 * j + 1],
                    op=AL.add,
                )
            u_ps = ps_mega.tile([P, 2 * TQ], F32, tag="mega", name="u")
            den_ps = ps_mega.tile([P, 2 * TQ], F32, tag="mega", name="den")
            u3 = u_ps.rearrange("p (h q) -> p h q", h=2)
            d3 = den_ps.rearrange("p (h q) -> p h q", h=2)
            for ik in range(nsk):
                last = ik == nsk - 1
                rhs = expst[:, ik, :, P:TQ] if last else expst[:, ik, :, :]
                nc.tensor.matmul(
                    u3[:, :, P:TQ] if last else u3,
                    vv[:, ik, :], rhs,
                    start=(ik == 0), stop=last,
                )
            for j in range(nsk // 2):
                nc.tensor.matmul(
                    d3, ones, esum[:, j],
                    start=(j == 0), stop=(j == nsk // 2 - 1),
                )
            rec = pb.tile([P, 2 * TQ], F32, tag="rec", bufs=2)
            nc.vector.reciprocal_approx_fast(out=rec, in_=den_ps)
            nc.vector.tensor_tensor(
                out=uT_t[:, 2 * hp : 2 * hp + 2, :],
                in0=u3,
                in1=rec.rearrange("p (h q) -> p h q", h=2),
                op=AL.mult,
            )

        def wo_stage(t, uT_t):
            for sub in range(2):
                g = 2 * t + sub
                y_sb = pb.tile([P, HID], F16, tag="ysb", bufs=2)
                for n in range(HID // 512):
                    y_ps = ps_mega.tile([P, 512], F32, tag="mega", name="y")
                    for h in range(NH):
                        nc.tensor.matmul(
                            y_ps,
                            uT_t[:, h, sub * P : (sub + 1) * P],
                            wo_sb[:, h, n * 512 : (n + 1) * 512],
                            start=(h == 0), stop=(h == NH - 1),
                        )
                    if n % 2 == 0:
                        nc.vector.tensor_copy(
                            y_sb[:, n * 512 : (n + 1) * 512], y_ps
                        )
                    else:
                        nc.scalar.activation(
                            out=y_sb[:, n * 512 : (n + 1) * 512], in_=y_ps,
                            func=AF.Copy,
                        )
                    nc.sync.dma_start(
                        out=out_d[g * P : (g + 1) * P, n * 512 : (n + 1) * 512],
                        in_=y_sb[:, n * 512 : (n + 1) * 512],
                    )

        # ================= driver =================
        ropes = [None] * NG
        qkvs = [None] * NG

        def emit_phase_a(g):
            if g in (3, 4):
                emit_wo_dma()
                emit_wo_dma()
            if g >= 1:
                gg = g - 1
                sc = nc.named_scope(f"rope_{gg}"); sc.__enter__()
                ropes[gg] = rope_stage(gg, qkvs[gg])
                sc.__exit__(None, None, None)
                sc = nc.named_scope(f"ropeT_{gg}"); sc.__enter__()
                rope_transpose(gg, *ropes[gg])
                sc.__exit__(None, None, None)
                ropes[gg] = None
                qkvs[gg] = None
            if g < NG:
                if g + 4 < NG:
                    emit_xdma(g + 4)
                sc = nc.named_scope(f"proj_{g}"); sc.__enter__()
                qkvs[g] = proj(g)
                sc.__exit__(None, None, None)

        steps = [(t, hp) for t in range(NT) for hp in range(2)]
        uts = {}
        att_i = [0]

        def emit_attention_step():
            # dnpv/wo (always PE-ready) go before the next scores step,
            # whose matmuls may still be blocked on ropeT of a later chunk.
            i = att_i[0]
            if i >= len(steps) + 1:
                return False
            if 1 <= i:
                t, hp = steps[i - 1]
                sc = nc.named_scope(f"dnpv_{t}_{hp}"); sc.__enter__()
                dnpv_step(t, hp, uts.pop((t, hp)), uts[t])
                sc.__exit__(None, None, None)
                if hp == 1:
                    sc = nc.named_scope(f"wo_{t}"); sc.__enter__()
                    wo_stage(t, uts.pop(t))
                    sc.__exit__(None, None, None)
            if i < len(steps):
                t, hp = steps[i]
                if hp == 0:
                    uts[t] = pb.tile([P, NH, TQ], F16, tag="uT", name=f"uT{t}")
                sc = nc.named_scope(f"sc_{t}_{hp}"); sc.__enter__()
                uts[(t, hp)] = scores_step(t, hp)
                sc.__exit__(None, None, None)
            att_i[0] += 1
            return True

        for g in range(NG + 1):
            emit_phase_a(g)
            done_g = g - 1  # ropeT for this chunk just emitted
            while att_i[0] < len(steps) + 1:
                i = att_i[0]
                if i < len(steps):
                    t, _hp = steps[i]
                    if 2 * t + 1 > done_g:
                        break
                emit_attention_step()
        while emit_attention_step():
            pass

    nc.compile()
    return nc


def shard_inputs(x, cos, sin, wq, wk, wv, wo):
    """Build per-core input maps: core = b*4 + g. All f16, pre-packed into
    the exact SBUF layouts so every DMA is contiguous per partition."""
    f16 = np.float16
    xts = []
    for b in range(B):
        xb = np.asarray(x[b], dtype=f16).reshape(NG, P, KC, P)
        xts.append(np.ascontiguousarray(xb.transpose(3, 0, 2, 1)).reshape(P, NG * KC * P))
    cs = np.concatenate(
        [
            np.asarray(cos, f16).reshape(NG, P, D).transpose(1, 0, 2).reshape(P, NG * D),
            np.asarray(sin, f16).reshape(NG, P, D).transpose(1, 0, 2).reshape(P, NG * D),
        ],
        axis=1,
    )
    cs = np.ascontiguousarray(cs)
    in_maps = []
    for c in range(N_CORES):
        b, g = divmod(c, N_KV)
        wq_g = np.asarray(wq[:, g * NH * D : (g + 1) * NH * D], f16)
        wq_p = np.ascontiguousarray(
            wq_g.reshape(KC, P, NH * D).transpose(1, 0, 2)
        ).reshape(P, KC * NH * D)
        wk_g = np.asarray(wk[:, g * D : (g + 1) * D], f16).reshape(KC, P, D)
        wv_g = np.asarray(wv[:, g * D : (g + 1) * D], f16).reshape(KC, P, D)
        wkv_p = np.ascontiguousarray(
            np.concatenate([wk_g, wv_g], axis=2).transpose(1, 0, 2)
        ).reshape(P, KC * 2 * D)
        wo_g = np.asarray(wo[g * NH * D : (g + 1) * NH * D, :], f16)
        wo_p = np.ascontiguousarray(
            wo_g.reshape(NH, P, HID).transpose(1, 0, 2)
        ).reshape(P, NH * HID)
        in_maps.append(
            {"xt": xts[b], "cs": cs, "wq": wq_p, "wkv": wkv_p, "wo": wo_p}
        )
    return in_maps


_NC_CACHE = {}


def get_nc():
    if "nc" not in _NC_CACHE:
        _NC_CACHE["nc"] = build_nc()
    return _NC_CACHE["nc"]


def kernel(x, cos, sin, wq, wk, wv, wo, _trace=False):
    from concourse.bass_utils import run_bass_kernel_spmd

    x = np.asarray(x, dtype=np.float32)
    cos = np.asarray(cos, dtype=np.float32)
    sin = np.asarray(sin, dtype=np.float32)
    wq = np.asarray(wq, dtype=np.float32)
    wk = np.asarray(wk, dtype=np.float32)
    wv = np.asarray(wv, dtype=np.float32)
    wo = np.asarray(wo, dtype=np.float32)

    nc = get_nc()
    in_maps = shard_inputs(x, cos, sin, wq, wk, wv, wo)
    res = run_bass_kernel_spmd(nc, in_maps, list(range(N_CORES)), trace=_trace)
    parts = [np.asarray(res.results[c]["out"], dtype=np.float32) for c in range(N_CORES)]
    y = np.stack(
        [sum(parts[b * N_KV + g] for g in range(N_KV)) for b in range(B)], axis=0
    )
    if _trace:
        kernel.last_result = res
    return y


# revision 50
# speedup vs baseline: 1.0041x; 1.0041x over previous
"""Trainium2 Bass kernel for GQA attention with RoPE (B=2, S=1024, HID=2048,
16 q heads / 4 kv heads, head dim 128, causal).

Sharding: 8 cores = 2 batches x 4 kv-head groups. Core c = b*4 + g handles
batch b and kv head g (query heads 4g..4g+3). Each core computes a partial
output y_part = attn_heads @ wo_shard; the host sums the 4 partials per batch.

v12 (final): all-f16 dataflow. Host pre-transposes x and packs every input
into SBUF layout so each DMA is a contiguous per-partition run (no on-device
x transposes; wk/wv previously moved as 256B strided runs, slowing the
fill). Softmax denominator matmuls are halved by pair-summing exp chunks on
the DVE inside each dnpv step, with the PV matmuls emitted first so the PE
covers the add latency. fp8 (incl. DoubleRow) was evaluated and rejected:
HW DoubleRow measures 1.0 cyc/row (2x FLOPs, not the cost model's 4x), and
e4m3's ~6% worst-case element error exceeds the 2e-2 max-error tolerance
through the softmax on every quantization point tested.
"""

import sys

import numpy as np

for _p in ("/opt/trn_rl_repo", "/root/.axon_site/_ro/trn_rl_repo"):
    if _p not in sys.path:
        sys.path.append(_p)

from contextlib import ExitStack

import concourse.bass as bass
import concourse.mybir as mybir
from concourse import bacc
from concourse.masks import make_identity
from concourse.tile import TileContext

P = 128           # partitions / head dim / seq chunk
S = 1024          # sequence length
HID = 2048        # model dim
NH = 4            # query heads per core
D = 128           # head dim
TQ = 256          # query macro-tile
NT = S // TQ      # 4 macro tiles
KC = HID // P     # 16 contraction chunks
NSK = S // P      # 8 key chunks
NG = S // P       # 8 row chunks
F16 = mybir.dt.float16
F32 = mybir.dt.float32
SCALE = 1.0 / float(np.sqrt(D))
AL = mybir.AluOpType
AF = mybir.ActivationFunctionType

N_CORES = 8
B = 2
N_KV = 4
H2 = D // 2


def build_nc(dbg=False):
    nc = bacc.Bacc("TRN2", target_bir_lowering=False, debug=False)
    # all inputs pre-packed by the host into SBUF layout (partition-major,
    # contiguous per-partition runs)
    xt_d = nc.declare_dram_parameter("xt", [P, NG * KC * P], F16, isOutput=False)
    cs_d = nc.declare_dram_parameter("cs", [P, 2 * NG * D], F16, isOutput=False)
    wq_d = nc.declare_dram_parameter("wq", [P, KC * NH * D], F16, isOutput=False)
    wkv_d = nc.declare_dram_parameter("wkv", [P, KC * 2 * D], F16, isOutput=False)
    wo_d = nc.declare_dram_parameter("wo", [P, NH * HID], F16, isOutput=False)
    out_d = nc.declare_dram_parameter("out", [S, HID], F16, isOutput=True)

    with TileContext(nc) as tc, ExitStack() as ctx:
        consts = ctx.enter_context(tc.tile_pool(name="consts", bufs=1))
        wpool = ctx.enter_context(tc.tile_pool(name="wpool", bufs=1))
        persist = ctx.enter_context(tc.tile_pool(name="persist", bufs=1))

        # ---- constants ----
        ident_f32 = consts.tile([P, P], F32, tag="ident_f32")
        make_identity(nc, ident_f32)
        ident = consts.tile([P, P], F16, tag="ident")
        nc.vector.tensor_copy(ident, ident_f32)
        ones = consts.tile([P, P], F16, tag="ones")
        nc.vector.memset(ones, 1.0)
        warm16 = consts.tile([P, 512], F16, tag="warm16")
        nc.vector.memset(warm16, 1.0)
        # causal 0/1 triangle: tri01[k, h, q] = (q >= k), f16, shared by both
        # diagonal chunks of every macro tile
        tri01 = consts.tile([P, 2, P], F16, tag="tri01")
        nc.gpsimd.memset(tri01, 1.0)
        nc.gpsimd.affine_select(
            out=tri01, in_=tri01, compare_op=AL.is_ge, fill=0.0,
            base=0, pattern=[[0, 2], [1, P]], channel_multiplier=-1,
        )

        # ---- persistent weights / tables / activations ----
        # wq in 4 quarter tiles: DMA gating is tile-granular, so quarters
        # let the first q matmuls start after 1MB of DMA instead of 2.5MB
        wq_sbq = [
            wpool.tile([P, 4, NH * D], F16, tag=f"wq{i}", name=f"wq_sbq{i}")
            for i in range(4)
        ]
        wkv_sb = wpool.tile([P, KC, 2 * D], F16, tag="wkv")
        wo_sb = wpool.tile([P, NH, HID], F16, tag="wo")
        cos_sb = wpool.tile([P, NG, D], F16, tag="cos")
        sin_sb = wpool.tile([P, NG, D], F16, tag="sin")

        qT_all = persist.tile([P, NH, S], F16, tag="qT")    # [d, h, sq]
        kT = persist.tile([P, S], F16, tag="kT")            # [d, sk]
        vv = persist.tile([P, NSK, D], F16, tag="vv")       # v natural [sk, d]

        # ---- pools ----
        pa = ctx.enter_context(tc.tile_pool(name="pa", bufs=2))
        pb = ctx.enter_context(tc.tile_pool(name="pb", bufs=2))
        ps_mega = ctx.enter_context(tc.tile_pool(name="ps_mega", bufs=7, space="PSUM"))
        ps_qkv = ctx.enter_context(tc.tile_pool(name="ps_qkv", bufs=1, space="PSUM"))

        # warm the PE clock gate while initial DMAs land
        warm_ps = ps_mega.tile([P, 512], F32, tag="mega", name="warm")
        for _ in range(8):
            nc.tensor.matmul(warm_ps[:, 0:P], ones, ones, start=True, stop=True)
        for _ in range(12):
            nc.tensor.matmul(warm_ps, ones, warm16, start=True, stop=True)
        warm_drain = pa.tile([P, 4], F32, tag="warmdrain", bufs=1)
        nc.vector.tensor_copy(warm_drain, warm_ps[:, 0:4])

        # ---- DMAs: xt per-chunk (sync queue), cos/sin on the scalar queue,
        # weights on the sync queue -- v4 order, packed layouts. ----
        x_tiles = [None] * NG

        def emit_xdma(g):
            # two half-tiles per chunk: DMA gating is tile-granular, so the
            # first q matmuls only wait on 0.25MB of x instead of 0.5MB
            xa = pa.tile([P, KC // 2, P], F16, tag="xTa", bufs=4, name=f"xa{g}")
            xb = pa.tile([P, KC // 2, P], F16, tag="xTb", bufs=4, name=f"xb{g}")
            half = KC * P // 2
            nc.sync.dma_start(
                out=xa.rearrange("p c d -> p (c d)"),
                in_=xt_d[:, g * KC * P : g * KC * P + half],
            )
            nc.sync.dma_start(
                out=xb.rearrange("p c d -> p (c d)"),
                in_=xt_d[:, g * KC * P + half : (g + 1) * KC * P],
            )
            x_tiles[g] = (xa, xb)

        def xt_c(g, c):
            return x_tiles[g][c // (KC // 2)][:, c % (KC // 2), :]

        nc.sync.dma_start(
            out=wq_sbq[0].rearrange("p c n -> p (c n)"),
            in_=wq_d[:, 0 : 4 * NH * D],
        )
        emit_xdma(0)
        nc.scalar.dma_start(
            out=cos_sb.rearrange("p g d -> p (g d)"), in_=cs_d[:, 0 : NG * D]
        )
        nc.scalar.dma_start(
            out=sin_sb.rearrange("p g d -> p (g d)"),
            in_=cs_d[:, NG * D : 2 * NG * D],
        )
        for i in range(1, 4):
            nc.sync.dma_start(
                out=wq_sbq[i].rearrange("p c n -> p (c n)"),
                in_=wq_d[:, i * 4 * NH * D : (i + 1) * 4 * NH * D],
            )
        nc.sync.dma_start(
            out=wkv_sb.rearrange("p c d -> p (c d)"), in_=wkv_d[:]
        )
        emit_xdma(1)
        emit_xdma(2)
        emit_xdma(3)
        wo_next = [0]

        def emit_wo_dma():
            h = wo_next[0]
            if h < NH:
                nc.sync.dma_start(
                    out=wo_sb[:, h, :], in_=wo_d[:, h * HID : (h + 1) * HID]
                )
                wo_next[0] += 1

        def bcast_h(ap2d, n):
            """[P, w] slice -> [P, n, w] broadcast AP (0-stride head dim)."""
            return ap2d.rearrange("p (o w) -> p o w", o=1).to_broadcast(
                [P, n, ap2d.shape[-1]]
            )

        # ================= phase A stages =================
        def proj(g):
            """q and kv projections for chunk g (PE, accumulating).
            q uses the dedicated 1-bank pool; kv borrows a mega slot so the
            attention phase gets a 7-deep mega rotation. For g=0 kv goes
            first: it only needs wkv+xt0 (1MB of DMA)."""
            q_ps = ps_qkv.tile([P, NH * D], F32, tag="qkv")
            kv_ps = ps_mega.tile([P, 512], F32, tag="mega", name="kv")[:, 0 : 2 * D]
            for c in range(KC):
                nc.tensor.matmul(
                    q_ps, xt_c(g, c), wq_sbq[c // 4][:, c % 4, :],
                    start=(c == 0), stop=(c == KC - 1),
                )
            for c in range(KC):
                nc.tensor.matmul(
                    kv_ps, xt_c(g, c), wkv_sb[:, c, :],
                    start=(c == 0), stop=(c == KC - 1),
                )
            qkv_sb = pa.tile([P, NH * D + 2 * D], F16, tag="qkvsb")
            nc.scalar.activation(out=qkv_sb[:, 0 : NH * D], in_=q_ps, func=AF.Copy)
            nc.scalar.activation(
                out=qkv_sb[:, NH * D : NH * D + 2 * D], in_=kv_ps, func=AF.Copy
            )
            return qkv_sb

        def rope_stage(g, qkv_sb):
            """RoPE on q heads (one 4-head strided pass) + k; v copy-out."""
            q3 = qkv_sb[:, 0 : NH * D].rearrange("p (h d) -> p h d", h=NH)
            k2 = qkv_sb[:, NH * D : NH * D + D]
            cos_g = cos_sb[:, g, :]
            sin_g = sin_sb[:, g, :]

            q_rope = pa.tile([P, NH, D], F16, tag="qrope")
            tmpq = pa.tile([P, NH, D], F16, tag="tmpq")
            nc.vector.scalar_tensor_tensor(
                out=tmpq[:, :, 0:H2], in0=q3[:, :, H2:D], scalar=-1.0,
                in1=bcast_h(sin_g[:, 0:H2], NH), op0=AL.mult, op1=AL.mult,
            )
            nc.vector.tensor_tensor(
                out=tmpq[:, :, H2:D], in0=q3[:, :, 0:H2],
                in1=bcast_h(sin_g[:, H2:D], NH), op=AL.mult,
            )
            nc.vector.tensor_tensor(
                out=q_rope, in0=q3, in1=bcast_h(cos_g, NH), op=AL.mult
            )
            nc.vector.tensor_tensor(out=q_rope, in0=q_rope, in1=tmpq, op=AL.add)

            k_rope = pa.tile([P, D], F16, tag="krope")
            tmpk = pa.tile([P, D], F16, tag="tmpk")
            nc.vector.scalar_tensor_tensor(
                out=tmpk[:, 0:H2], in0=k2[:, H2:D], scalar=-1.0,
                in1=sin_g[:, 0:H2], op0=AL.mult, op1=AL.mult,
            )
            nc.vector.tensor_tensor(
                out=tmpk[:, H2:D], in0=k2[:, 0:H2], in1=sin_g[:, H2:D], op=AL.mult
            )
            nc.vector.tensor_tensor(out=k_rope, in0=k2, in1=cos_g, op=AL.mult)
            nc.vector.tensor_tensor(out=k_rope, in0=k_rope, in1=tmpk, op=AL.add)

            nc.vector.tensor_copy(
                vv[:, g, :], qkv_sb[:, NH * D + D : NH * D + 2 * D]
            )
            return q_rope, k_rope

        def rope_transpose(g, q_rope, k_rope):
            """Transpose RoPE'd q/k into persistent qT_all / kT (PE, f16)."""
            tq_ps = ps_mega.tile([P, 4 * P], F16, tag="mega", name="tq")
            for h in range(NH):
                nc.tensor.transpose(
                    tq_ps[:, h * P : (h + 1) * P], q_rope[:, h, :], ident
                )
            nc.vector.tensor_copy(
                qT_all[:, :, g * P : (g + 1) * P],
                tq_ps.rearrange("p (h d) -> p h d", h=NH),
            )
            tk_ps = ps_mega.tile([P, 4 * P], F16, tag="mega", name="tk")
            nc.tensor.transpose(tk_ps[:, 0:P], k_rope, ident)
            nc.vector.tensor_copy(kT[:, g * P : (g + 1) * P], tk_ps[:, 0:P])

        # ================= phase B stages =================
        def scores_step(t, hp):
            """scoresT + exp for head-pair hp of macro tile t -> expst.

            expst[sk, ik, h2, q]: per key chunk ik, both heads of the pair.
            Diagonal chunks get a post-exp 0/1 triangle multiply; the odd
            diagonal chunk only computes the upper query half."""
            q0 = t * TQ
            nsk = 2 * (t + 1)
            expst = pb.tile([P, NSK, 2, TQ], F16, tag="expst", bufs=2)
            for ik in range(nsk):
                s_ps = ps_mega.tile([P, 2 * TQ], F32, tag="mega", name="s")
                s3 = s_ps.rearrange("p (h q) -> p h q", h=2)
                if ik == nsk - 1:  # odd diagonal: queries q0+128..q0+255 only
                    nc.gpsimd.memset(expst[:, ik, :, 0:P], 0.0)
                    nc.tensor.matmul(
                        s3[:, :, P:TQ],
                        kT[:, ik * P : (ik + 1) * P],
                        qT_all[:, 2 * hp : 2 * hp + 2, q0 + P : q0 + TQ],
                        start=True, stop=True,
                    )
                    nc.scalar.activation(
                        out=expst[:, ik, :, P:TQ], in_=s3[:, :, P:TQ],
                        func=AF.Exp, scale=SCALE,
                    )
                    nc.vector.tensor_tensor(
                        out=expst[:, ik, :, P:TQ], in0=expst[:, ik, :, P:TQ],
                        in1=tri01, op=AL.mult,
                    )
                elif t == NT - 1 and hp == 0:
                    # split into q-chunk halves: the first half only needs
                    # ropeT(2t), so the PE isn't gated on the last ropeT
                    nc.tensor.matmul(
                        s3[:, :, 0:P],
                        kT[:, ik * P : (ik + 1) * P],
                        qT_all[:, 0:2, q0 : q0 + P],
                        start=True, stop=True, skip_group_check=True,
                    )
                    nc.tensor.matmul(
                        s3[:, :, P:TQ],
                        kT[:, ik * P : (ik + 1) * P],
                        qT_all[:, 0:2, q0 + P : q0 + TQ],
                        start=True, stop=True, skip_group_check=True,
                    )
                    nc.scalar.activation(
                        out=expst[:, ik, :, :], in_=s3, func=AF.Exp, scale=SCALE,
                    )
                    if ik == nsk - 2:  # even diagonal: lower-left triangle
                        nc.vector.tensor_tensor(
                            out=expst[:, ik, :, 0:P], in0=expst[:, ik, :, 0:P],
                            in1=tri01, op=AL.mult,
                        )
                else:
                    nc.tensor.matmul(
                        s3,
                        kT[:, ik * P : (ik + 1) * P],
                        qT_all[:, 2 * hp : 2 * hp + 2, q0 : q0 + TQ],
                        start=True, stop=True,
                    )
                    nc.scalar.activation(
                        out=expst[:, ik, :, :], in_=s3, func=AF.Exp, scale=SCALE,
                    )
                    if ik == nsk - 2:  # even diagonal: lower-left triangle
                        nc.vector.tensor_tensor(
                            out=expst[:, ik, :, 0:P], in0=expst[:, ik, :, 0:P],
                            in1=tri01, op=AL.mult,
                        )
            return expst

        def dnpv_step(t, hp, expst, uT_t):
            """DVE pair-sums + PV matmuls + halved denominator matmuls,
            then normalize into uT_t. PV runs while the DVE adds complete."""
            nsk = 2 * (t + 1)
            esum = pb.tile([P, NSK // 2, 2, TQ], F16, tag="esum", bufs=2)
            for j in range(nsk // 2):
                nc.vector.tensor_tensor(
                    out=esum[:, j], in0=expst[:, 2 * j], in1=expst[:, 2 * j + 1],
                    op=AL.add,
                )
            u_ps = ps_mega.tile([P, 2 * TQ], F32, tag="mega", name="u")
            den_ps = ps_mega.tile([P, 2 * TQ], F32, tag="mega", name="den")
            u3 = u_ps.rearrange("p (h q) -> p h q", h=2)
            d3 = den_ps.rearrange("p (h q) -> p h q", h=2)
            for ik in range(nsk):
                last = ik == nsk - 1
                rhs = expst[:, ik, :, P:TQ] if last else expst[:, ik, :, :]
                nc.tensor.matmul(
                    u3[:, :, P:TQ] if last else u3,
                    vv[:, ik, :], rhs,
                    start=(ik == 0), stop=last,
                )
            for j in range(nsk // 2):
                nc.tensor.matmul(
                    d3, ones, esum[:, j],
                    start=(j == 0), stop=(j == nsk // 2 - 1),
                )
            rec = pb.tile([P, 2 * TQ], F32, tag="rec", bufs=2)
            nc.vector.reciprocal_approx_fast(out=rec, in_=den_ps)
            nc.vector.tensor_tensor(
                out=uT_t[:, 2 * hp : 2 * hp + 2, :],
                in0=u3,
                in1=rec.rearrange("p (h q) -> p h q", h=2),
                op=AL.mult,
            )

        def wo_stage(t, uT_t):
            for sub in range(2):
                g = 2 * t + sub
                y_sb = pb.tile([P, HID], F16, tag="ysb", bufs=2)
                for n in range(HID // 512):
                    y_ps = ps_mega.tile([P, 512], F32, tag="mega", name="y")
                    for h in range(NH):
                        nc.tensor.matmul(
                            y_ps,
                            uT_t[:, h, sub * P : (sub + 1) * P],
                            wo_sb[:, h, n * 512 : (n + 1) * 512],
                            start=(h == 0), stop=(h == NH - 1),
                        )
                    if n % 2 == 0:
                        nc.vector.tensor_copy(
                            y_sb[:, n * 512 : (n + 1) * 512], y_ps
                        )
                    else:
                        nc.scalar.activation(
                            out=y_sb[:, n * 512 : (n + 1) * 512], in_=y_ps,
                            func=AF.Copy,
                        )
                    nc.sync.dma_start(
                        out=out_d[g * P : (g + 1) * P, n * 512 : (n + 1) * 512],
                        in_=y_sb[:, n * 512 : (n + 1) * 512],
                    )

        # ================= driver =================
        ropes = [None] * NG
        qkvs = [None] * NG

        def emit_phase_a(g):
            if g in (3, 4):
                emit_wo_dma()
                emit_wo_dma()
            if g >= 1:
                gg = g - 1
                sc = nc.named_scope(f"rope_{gg}"); sc.__enter__()
                ropes[gg] = rope_stage(gg, qkvs[gg])
                sc.__exit__(None, None, None)
                sc = nc.named_scope(f"ropeT_{gg}"); sc.__enter__()
                rope_transpose(gg, *ropes[gg])
                sc.__exit__(None, None, None)
                ropes[gg] = None
                qkvs[gg] = None
            if g < NG:
                if g + 4 < NG:
                    emit_xdma(g + 4)
                sc = nc.named_scope(f"proj_{g}"); sc.__enter__()
                qkvs[g] = proj(g)
                sc.__exit__(None, None, None)

        steps = [(t, hp) for t in range(NT) for hp in range(2)]
        uts = {}
        att_i = [0]

        def emit_attention_step():
            # dnpv/wo (always PE-ready) go before the next scores step,
            # whose matmuls may still be blocked on ropeT of a later chunk.
            i = att_i[0]
            if i >= len(steps) + 1:
                return False
            if 1 <= i:
                t, hp = steps[i - 1]
                sc = nc.named_scope(f"dnpv_{t}_{hp}"); sc.__enter__()
                dnpv_step(t, hp, uts.pop((t, hp)), uts[t])
                sc.__exit__(None, None, None)
                if hp == 1:
                    sc = nc.named_scope(f"wo_{t}"); sc.__enter__()
                    wo_stage(t, uts.pop(t))
                    sc.__exit__(None, None, None)
            if i < len(steps):
                t, hp = steps[i]
                if hp == 0:
                    uts[t] = pb.tile([P, NH, TQ], F16, tag="uT", name=f"uT{t}")
                sc = nc.named_scope(f"sc_{t}_{hp}"); sc.__enter__()
                uts[(t, hp)] = scores_step(t, hp)
                sc.__exit__(None, None, None)
            att_i[0] += 1
            return True

        for g in range(NG + 1):
            emit_phase_a(g)
            done_g = g - 1  # ropeT for this chunk just emitted
            while att_i[0] < len(steps) + 1:
                i = att_i[0]
                if i < len(steps):
                    t, _hp = steps[i]
                    if 2 * t + 1 > done_g:
                        break
                emit_attention_step()
        while emit_attention_step():
            pass

    nc.compile()
    return nc


def shard_inputs(x, cos, sin, wq, wk, wv, wo):
    """Build per-core input maps: core = b*4 + g. All f16, pre-packed into
    the exact SBUF layouts so every DMA is contiguous per partition."""
    f16 = np.float16
    xts = []
    for b in range(B):
        xb = np.asarray(x[b], dtype=f16).reshape(NG, P, KC, P)
        xts.append(np.ascontiguousarray(xb.transpose(3, 0, 2, 1)).reshape(P, NG * KC * P))
    cs = np.concatenate(
        [
            np.asarray(cos, f16).reshape(NG, P, D).transpose(1, 0, 2).reshape(P, NG * D),
            np.asarray(sin, f16).reshape(NG, P, D).transpose(1, 0, 2).reshape(P, NG * D),
        ],
        axis=1,
    )
    cs = np.ascontiguousarray(cs)
    in_maps = []
    for c in range(N_CORES):
        b, g = divmod(c, N_KV)
        wq_g = np.asarray(wq[:, g * NH * D : (g + 1) * NH * D], f16)
        wq_p = np.ascontiguousarray(
            wq_g.reshape(KC, P, NH * D).transpose(1, 0, 2)
        ).reshape(P, KC * NH * D)
        wk_g = np.asarray(wk[:, g * D : (g + 1) * D], f16).reshape(KC, P, D)
        wv_g = np.asarray(wv[:, g * D : (g + 1) * D], f16).reshape(KC, P, D)
        wkv_p = np.ascontiguousarray(
            np.concatenate([wk_g, wv_g], axis=2).transpose(1, 0, 2)
        ).reshape(P, KC * 2 * D)
        wo_g = np.asarray(wo[g * NH * D : (g + 1) * NH * D, :], f16)
        wo_p = np.ascontiguousarray(
            wo_g.reshape(NH, P, HID).transpose(1, 0, 2)
        ).reshape(P, NH * HID)
        in_maps.append(
            {"xt": xts[b], "cs": cs, "wq": wq_p, "wkv": wkv_p, "wo": wo_p}
        )
    return in_maps


_NC_CACHE = {}


def get_nc():
    if "nc" not in _NC_CACHE:
        _NC_CACHE["nc"] = build_nc()
    return _NC_CACHE["nc"]


def kernel(x, cos, sin, wq, wk, wv, wo, _trace=False):
    from concourse.bass_utils import run_bass_kernel_spmd

    x = np.asarray(x, dtype=np.float32)
    cos = np.asarray(cos, dtype=np.float32)
    sin = np.asarray(sin, dtype=np.float32)
    wq = np.asarray(wq, dtype=np.float32)
    wk = np.asarray(wk, dtype=np.float32)
    wv = np.asarray(wv, dtype=np.float32)
    wo = np.asarray(wo, dtype=np.float32)

    nc = get_nc()
    in_maps = shard_inputs(x, cos, sin, wq, wk, wv, wo)
    res = run_bass_kernel_spmd(nc, in_maps, list(range(N_CORES)), trace=_trace)
    parts = [np.asarray(res.results[c]["out"], dtype=np.float32) for c in range(N_CORES)]
    y = np.stack(
        [sum(parts[b * N_KV + g] for g in range(N_KV)) for b in range(B)], axis=0
    )
    if _trace:
        kernel.last_result = res
    return y


# revision 51
# speedup vs baseline: 1.0205x; 1.0163x over previous
"""Trainium2 Bass kernel for GQA attention with RoPE (B=2, S=1024, HID=2048,
16 q heads / 4 kv heads, head dim 128, causal).

Sharding: 8 cores = 2 batches x 4 kv-head groups. Core c = b*4 + g handles
batch b and kv head g (query heads 4g..4g+3). Each core computes a partial
output y_part = attn_heads @ wo_shard; the host sums the 4 partials per batch.

v12 (final): all-f16 dataflow. Host pre-transposes x and packs every input
into SBUF layout so each DMA is a contiguous per-partition run (no on-device
x transposes; wk/wv previously moved as 256B strided runs, slowing the
fill). Softmax denominator matmuls are halved by pair-summing exp chunks on
the DVE inside each dnpv step, with the PV matmuls emitted first so the PE
covers the add latency. fp8 (incl. DoubleRow) was evaluated and rejected:
HW DoubleRow measures 1.0 cyc/row (2x FLOPs, not the cost model's 4x), and
e4m3's ~6% worst-case element error exceeds the 2e-2 max-error tolerance
through the softmax on every quantization point tested.
"""

import sys

import numpy as np

for _p in ("/opt/trn_rl_repo", "/root/.axon_site/_ro/trn_rl_repo"):
    if _p not in sys.path:
        sys.path.append(_p)

from contextlib import ExitStack

import concourse.bass as bass
import concourse.mybir as mybir
from concourse import bacc
from concourse.masks import make_identity
from concourse.tile import TileContext

P = 128           # partitions / head dim / seq chunk
S = 1024          # sequence length
HID = 2048        # model dim
NH = 4            # query heads per core
D = 128           # head dim
TQ = 256          # query macro-tile
NT = S // TQ      # 4 macro tiles
KC = HID // P     # 16 contraction chunks
NSK = S // P      # 8 key chunks
NG = S // P       # 8 row chunks
F16 = mybir.dt.float16
F32 = mybir.dt.float32
SCALE = 1.0 / float(np.sqrt(D))
AL = mybir.AluOpType
AF = mybir.ActivationFunctionType

N_CORES = 8
B = 2
N_KV = 4
H2 = D // 2


def build_nc(dbg=False):
    nc = bacc.Bacc("TRN2", target_bir_lowering=False, debug=False)
    # all inputs pre-packed by the host into SBUF layout (partition-major,
    # contiguous per-partition runs)
    xt_d = nc.declare_dram_parameter("xt", [P, NG * KC * P], F16, isOutput=False)
    cs_d = nc.declare_dram_parameter("cs", [P, 2 * NG * D], F16, isOutput=False)
    wq_d = nc.declare_dram_parameter("wq", [P, KC * NH * D], F16, isOutput=False)
    wkv_d = nc.declare_dram_parameter("wkv", [P, KC * 2 * D], F16, isOutput=False)
    wo_d = nc.declare_dram_parameter("wo", [P, NH * HID], F16, isOutput=False)
    out_d = nc.declare_dram_parameter("out", [S, HID], F16, isOutput=True)

    with TileContext(nc) as tc, ExitStack() as ctx:
        consts = ctx.enter_context(tc.tile_pool(name="consts", bufs=1))
        wpool = ctx.enter_context(tc.tile_pool(name="wpool", bufs=1))
        persist = ctx.enter_context(tc.tile_pool(name="persist", bufs=1))

        # ---- constants ----
        ident_f32 = consts.tile([P, P], F32, tag="ident_f32")
        make_identity(nc, ident_f32)
        ident = consts.tile([P, P], F16, tag="ident")
        nc.vector.tensor_copy(ident, ident_f32)
        ones = consts.tile([P, P], F16, tag="ones")
        nc.vector.memset(ones, 1.0)
        warm16 = consts.tile([P, 512], F16, tag="warm16")
        nc.vector.memset(warm16, 1.0)
        # causal 0/1 triangle: tri01[k, h, q] = (q >= k), f16, shared by both
        # diagonal chunks of every macro tile
        tri01 = consts.tile([P, 2, P], F16, tag="tri01")
        nc.gpsimd.memset(tri01, 1.0)
        nc.gpsimd.affine_select(
            out=tri01, in_=tri01, compare_op=AL.is_ge, fill=0.0,
            base=0, pattern=[[0, 2], [1, P]], channel_multiplier=-1,
        )

        # ---- persistent weights / tables / activations ----
        # wq in 4 quarter tiles: DMA gating is tile-granular, so quarters
        # let the first q matmuls start after 1MB of DMA instead of 2.5MB
        wq_sbq = [
            wpool.tile([P, 4, NH * D], F16, tag=f"wq{i}", name=f"wq_sbq{i}")
            for i in range(4)
        ]
        wkv_sb = wpool.tile([P, KC, 2 * D], F16, tag="wkv")
        wo_sb = wpool.tile([P, NH, HID], F16, tag="wo")
        cos_sb = wpool.tile([P, NG, D], F16, tag="cos")
        sin_sb = wpool.tile([P, NG, D], F16, tag="sin")

        qT_all = persist.tile([P, NH, S], F16, tag="qT")    # [d, h, sq]
        kT = persist.tile([P, S], F16, tag="kT")            # [d, sk]
        vv = persist.tile([P, NSK, D], F16, tag="vv")       # v natural [sk, d]

        # ---- pools ----
        pa = ctx.enter_context(tc.tile_pool(name="pa", bufs=2))
        pb = ctx.enter_context(tc.tile_pool(name="pb", bufs=2))
        ps_mega = ctx.enter_context(tc.tile_pool(name="ps_mega", bufs=7, space="PSUM"))
        ps_qkv = ctx.enter_context(tc.tile_pool(name="ps_qkv", bufs=1, space="PSUM"))

        # warm the PE clock gate while initial DMAs land
        warm_ps = ps_mega.tile([P, 512], F32, tag="mega", name="warm")
        for _ in range(8):
            nc.tensor.matmul(warm_ps[:, 0:P], ones, ones, start=True, stop=True)
        for _ in range(12):
            nc.tensor.matmul(warm_ps, ones, warm16, start=True, stop=True)
        warm_drain = pa.tile([P, 4], F32, tag="warmdrain", bufs=1)
        nc.vector.tensor_copy(warm_drain, warm_ps[:, 0:4])

        # ---- DMAs: xt per-chunk (sync queue), cos/sin on the scalar queue,
        # weights on the sync queue -- v4 order, packed layouts. ----
        x_tiles = [None] * NG

        def emit_xdma(g):
            # two half-tiles per chunk: DMA gating is tile-granular, so the
            # first q matmuls only wait on 0.25MB of x instead of 0.5MB
            xa = pa.tile([P, KC // 2, P], F16, tag="xTa", bufs=4, name=f"xa{g}")
            xb = pa.tile([P, KC // 2, P], F16, tag="xTb", bufs=4, name=f"xb{g}")
            half = KC * P // 2
            nc.sync.dma_start(
                out=xa.rearrange("p c d -> p (c d)"),
                in_=xt_d[:, g * KC * P : g * KC * P + half],
            )
            nc.sync.dma_start(
                out=xb.rearrange("p c d -> p (c d)"),
                in_=xt_d[:, g * KC * P + half : (g + 1) * KC * P],
            )
            x_tiles[g] = (xa, xb)

        def xt_c(g, c):
            return x_tiles[g][c // (KC // 2)][:, c % (KC // 2), :]

        nc.sync.dma_start(
            out=wq_sbq[0].rearrange("p c n -> p (c n)"),
            in_=wq_d[:, 0 : 4 * NH * D],
        )
        emit_xdma(0)
        nc.scalar.dma_start(
            out=cos_sb.rearrange("p g d -> p (g d)"), in_=cs_d[:, 0 : NG * D]
        )
        nc.scalar.dma_start(
            out=sin_sb.rearrange("p g d -> p (g d)"),
            in_=cs_d[:, NG * D : 2 * NG * D],
        )
        for i in range(1, 3):
            nc.sync.dma_start(
                out=wq_sbq[i].rearrange("p c n -> p (c n)"),
                in_=wq_d[:, i * 4 * NH * D : (i + 1) * 4 * NH * D],
            )
        # wkv before the last wq quarter: kv matmuls fill the wq3 DMA wait
        nc.sync.dma_start(
            out=wkv_sb.rearrange("p c d -> p (c d)"), in_=wkv_d[:]
        )
        nc.sync.dma_start(
            out=wq_sbq[3].rearrange("p c n -> p (c n)"),
            in_=wq_d[:, 3 * 4 * NH * D : 4 * 4 * NH * D],
        )
        emit_xdma(1)
        emit_xdma(2)
        emit_xdma(3)
        wo_next = [0]

        def emit_wo_dma():
            h = wo_next[0]
            if h < NH:
                nc.sync.dma_start(
                    out=wo_sb[:, h, :], in_=wo_d[:, h * HID : (h + 1) * HID]
                )
                wo_next[0] += 1

        def bcast_h(ap2d, n):
            """[P, w] slice -> [P, n, w] broadcast AP (0-stride head dim)."""
            return ap2d.rearrange("p (o w) -> p o w", o=1).to_broadcast(
                [P, n, ap2d.shape[-1]]
            )

        # ================= phase A stages =================
        def proj(g):
            """q and kv projections for chunk g (PE, accumulating).
            q uses the dedicated 1-bank pool; kv borrows a mega slot so the
            attention phase gets a 7-deep mega rotation. For g=0 kv goes
            first: it only needs wkv+xt0 (1MB of DMA)."""
            q_ps = ps_qkv.tile([P, NH * D], F32, tag="qkv")
            kv_ps = ps_mega.tile([P, 512], F32, tag="mega", name="kv")[:, 0 : 2 * D]
            for c in range(KC):
                nc.tensor.matmul(
                    q_ps, xt_c(g, c), wq_sbq[c // 4][:, c % 4, :],
                    start=(c == 0), stop=(c == KC - 1),
                )
            for c in range(KC):
                nc.tensor.matmul(
                    kv_ps, xt_c(g, c), wkv_sb[:, c, :],
                    start=(c == 0), stop=(c == KC - 1),
                )
            qkv_sb = pa.tile([P, NH * D + 2 * D], F16, tag="qkvsb")
            nc.scalar.activation(out=qkv_sb[:, 0 : NH * D], in_=q_ps, func=AF.Copy)
            nc.scalar.activation(
                out=qkv_sb[:, NH * D : NH * D + 2 * D], in_=kv_ps, func=AF.Copy
            )
            return qkv_sb

        def rope_stage(g, qkv_sb):
            """RoPE on q heads (one 4-head strided pass) + k; v copy-out."""
            q3 = qkv_sb[:, 0 : NH * D].rearrange("p (h d) -> p h d", h=NH)
            k2 = qkv_sb[:, NH * D : NH * D + D]
            cos_g = cos_sb[:, g, :]
            sin_g = sin_sb[:, g, :]

            q_rope = pa.tile([P, NH, D], F16, tag="qrope")
            tmpq = pa.tile([P, NH, D], F16, tag="tmpq")
            nc.vector.scalar_tensor_tensor(
                out=tmpq[:, :, 0:H2], in0=q3[:, :, H2:D], scalar=-1.0,
                in1=bcast_h(sin_g[:, 0:H2], NH), op0=AL.mult, op1=AL.mult,
            )
            nc.vector.tensor_tensor(
                out=tmpq[:, :, H2:D], in0=q3[:, :, 0:H2],
                in1=bcast_h(sin_g[:, H2:D], NH), op=AL.mult,
            )
            nc.vector.tensor_tensor(
                out=q_rope, in0=q3, in1=bcast_h(cos_g, NH), op=AL.mult
            )
            nc.vector.tensor_tensor(out=q_rope, in0=q_rope, in1=tmpq, op=AL.add)

            k_rope = pa.tile([P, D], F16, tag="krope")
            tmpk = pa.tile([P, D], F16, tag="tmpk")
            nc.vector.scalar_tensor_tensor(
                out=tmpk[:, 0:H2], in0=k2[:, H2:D], scalar=-1.0,
                in1=sin_g[:, 0:H2], op0=AL.mult, op1=AL.mult,
            )
            nc.vector.tensor_tensor(
                out=tmpk[:, H2:D], in0=k2[:, 0:H2], in1=sin_g[:, H2:D], op=AL.mult
            )
            nc.vector.tensor_tensor(out=k_rope, in0=k2, in1=cos_g, op=AL.mult)
            nc.vector.tensor_tensor(out=k_rope, in0=k_rope, in1=tmpk, op=AL.add)

            nc.vector.tensor_copy(
                vv[:, g, :], qkv_sb[:, NH * D + D : NH * D + 2 * D]
            )
            return q_rope, k_rope

        def rope_transpose(g, q_rope, k_rope):
            """Transpose RoPE'd q/k into persistent qT_all / kT (PE, f16)."""
            tq_ps = ps_mega.tile([P, 4 * P], F16, tag="mega", name="tq")
            for h in range(NH):
                nc.tensor.transpose(
                    tq_ps[:, h * P : (h + 1) * P], q_rope[:, h, :], ident
                )
            nc.vector.tensor_copy(
                qT_all[:, :, g * P : (g + 1) * P],
                tq_ps.rearrange("p (h d) -> p h d", h=NH),
            )
            tk_ps = ps_mega.tile([P, 4 * P], F16, tag="mega", name="tk")
            nc.tensor.transpose(tk_ps[:, 0:P], k_rope, ident)
            nc.vector.tensor_copy(kT[:, g * P : (g + 1) * P], tk_ps[:, 0:P])

        # ================= phase B stages =================
        def scores_step(t, hp):
            """scoresT + exp for head-pair hp of macro tile t -> expst.

            expst[sk, ik, h2, q]: per key chunk ik, both heads of the pair.
            Diagonal chunks get a post-exp 0/1 triangle multiply; the odd
            diagonal chunk only computes the upper query half."""
            q0 = t * TQ
            nsk = 2 * (t + 1)
            expst = pb.tile([P, NSK, 2, TQ], F16, tag="expst", bufs=2)
            for ik in range(nsk):
                s_ps = ps_mega.tile([P, 2 * TQ], F32, tag="mega", name="s")
                s3 = s_ps.rearrange("p (h q) -> p h q", h=2)
                if ik == nsk - 1:  # odd diagonal: queries q0+128..q0+255 only
                    nc.gpsimd.memset(expst[:, ik, :, 0:P], 0.0)
                    nc.tensor.matmul(
                        s3[:, :, P:TQ],
                        kT[:, ik * P : (ik + 1) * P],
                        qT_all[:, 2 * hp : 2 * hp + 2, q0 + P : q0 + TQ],
                        start=True, stop=True,
                    )
                    nc.scalar.activation(
                        out=expst[:, ik, :, P:TQ], in_=s3[:, :, P:TQ],
                        func=AF.Exp, scale=SCALE,
                    )
                    nc.vector.tensor_tensor(
                        out=expst[:, ik, :, P:TQ], in0=expst[:, ik, :, P:TQ],
                        in1=tri01, op=AL.mult,
                    )
                elif t == NT - 1 and hp == 0:
                    # split into q-chunk halves: the first half only needs
                    # ropeT(2t), so the PE isn't gated on the last ropeT
                    nc.tensor.matmul(
                        s3[:, :, 0:P],
                        kT[:, ik * P : (ik + 1) * P],
                        qT_all[:, 0:2, q0 : q0 + P],
                        start=True, stop=True, skip_group_check=True,
                    )
                    nc.tensor.matmul(
                        s3[:, :, P:TQ],
                        kT[:, ik * P : (ik + 1) * P],
                        qT_all[:, 0:2, q0 + P : q0 + TQ],
                        start=True, stop=True, skip_group_check=True,
                    )
                    nc.scalar.activation(
                        out=expst[:, ik, :, :], in_=s3, func=AF.Exp, scale=SCALE,
                    )
                    if ik == nsk - 2:  # even diagonal: lower-left triangle
                        nc.vector.tensor_tensor(
                            out=expst[:, ik, :, 0:P], in0=expst[:, ik, :, 0:P],
                            in1=tri01, op=AL.mult,
                        )
                else:
                    nc.tensor.matmul(
                        s3,
                        kT[:, ik * P : (ik + 1) * P],
                        qT_all[:, 2 * hp : 2 * hp + 2, q0 : q0 + TQ],
                        start=True, stop=True,
                    )
                    nc.scalar.activation(
                        out=expst[:, ik, :, :], in_=s3, func=AF.Exp, scale=SCALE,
                    )
                    if ik == nsk - 2:  # even diagonal: lower-left triangle
                        nc.vector.tensor_tensor(
                            out=expst[:, ik, :, 0:P], in0=expst[:, ik, :, 0:P],
                            in1=tri01, op=AL.mult,
                        )
            return expst

        def dnpv_step(t, hp, expst, uT_t):
            """DVE pair-sums + PV matmuls + halved denominator matmuls,
            then normalize into uT_t. PV runs while the DVE adds complete."""
            nsk = 2 * (t + 1)
            esum = pb.tile([P, NSK // 2, 2, TQ], F16, tag="esum", bufs=2)
            for j in range(nsk // 2):
                nc.vector.tensor_tensor(
                    out=esum[:, j], in0=expst[:, 2 * j], in1=expst[:, 2 * j + 1],
                    op=AL.add,
                )
            u_ps = ps_mega.tile([P, 2 * TQ], F32, tag="mega", name="u")
            den_ps = ps_mega.tile([P, 2 * TQ], F32, tag="mega", name="den")
            u3 = u_ps.rearrange("p (h q) -> p h q", h=2)
            d3 = den_ps.rearrange("p (h q) -> p h q", h=2)
            for ik in range(nsk):
                last = ik == nsk - 1
                rhs = expst[:, ik, :, P:TQ] if last else expst[:, ik, :, :]
                nc.tensor.matmul(
                    u3[:, :, P:TQ] if last else u3,
                    vv[:, ik, :], rhs,
                    start=(ik == 0), stop=last,
                )
            for j in range(nsk // 2):
                nc.tensor.matmul(
                    d3, ones, esum[:, j],
                    start=(j == 0), stop=(j == nsk // 2 - 1),
                )
            rec = pb.tile([P, 2 * TQ], F32, tag="rec", bufs=2)
            nc.vector.reciprocal_approx_fast(out=rec, in_=den_ps)
            nc.vector.tensor_tensor(
                out=uT_t[:, 2 * hp : 2 * hp + 2, :],
                in0=u3,
                in1=rec.rearrange("p (h q) -> p h q", h=2),
                op=AL.mult,
            )

        def wo_stage(t, uT_t):
            for sub in range(2):
                g = 2 * t + sub
                y_sb = pb.tile([P, HID], F16, tag="ysb", bufs=2)
                for n in range(HID // 512):
                    y_ps = ps_mega.tile([P, 512], F32, tag="mega", name="y")
                    for h in range(NH):
                        nc.tensor.matmul(
                            y_ps,
                            uT_t[:, h, sub * P : (sub + 1) * P],
                            wo_sb[:, h, n * 512 : (n + 1) * 512],
                            start=(h == 0), stop=(h == NH - 1),
                        )
                    if n % 2 == 0:
                        nc.vector.tensor_copy(
                            y_sb[:, n * 512 : (n + 1) * 512], y_ps
                        )
                    else:
                        nc.scalar.activation(
                            out=y_sb[:, n * 512 : (n + 1) * 512], in_=y_ps,
                            func=AF.Copy,
                        )
                    nc.sync.dma_start(
                        out=out_d[g * P : (g + 1) * P, n * 512 : (n + 1) * 512],
                        in_=y_sb[:, n * 512 : (n + 1) * 512],
                    )

        # ================= driver =================
        ropes = [None] * NG
        qkvs = [None] * NG

        def emit_phase_a(g):
            if g in (3, 4):
                emit_wo_dma()
                emit_wo_dma()
            if g >= 1:
                gg = g - 1
                sc = nc.named_scope(f"rope_{gg}"); sc.__enter__()
                ropes[gg] = rope_stage(gg, qkvs[gg])
                sc.__exit__(None, None, None)
                sc = nc.named_scope(f"ropeT_{gg}"); sc.__enter__()
                rope_transpose(gg, *ropes[gg])
                sc.__exit__(None, None, None)
                ropes[gg] = None
                qkvs[gg] = None
            if g < NG:
                if g + 4 < NG:
                    emit_xdma(g + 4)
                sc = nc.named_scope(f"proj_{g}"); sc.__enter__()
                qkvs[g] = proj(g)
                sc.__exit__(None, None, None)

        steps = [(t, hp) for t in range(NT) for hp in range(2)]
        uts = {}
        att_i = [0]

        def emit_attention_step():
            # dnpv/wo (always PE-ready) go before the next scores step,
            # whose matmuls may still be blocked on ropeT of a later chunk.
            i = att_i[0]
            if i >= len(steps) + 1:
                return False
            if 1 <= i:
                t, hp = steps[i - 1]
                sc = nc.named_scope(f"dnpv_{t}_{hp}"); sc.__enter__()
                dnpv_step(t, hp, uts.pop((t, hp)), uts[t])
                sc.__exit__(None, None, None)
                if hp == 1:
                    sc = nc.named_scope(f"wo_{t}"); sc.__enter__()
                    wo_stage(t, uts.pop(t))
                    sc.__exit__(None, None, None)
            if i < len(steps):
                t, hp = steps[i]
                if hp == 0:
                    uts[t] = pb.tile([P, NH, TQ], F16, tag="uT", name=f"uT{t}")
                sc = nc.named_scope(f"sc_{t}_{hp}"); sc.__enter__()
                uts[(t, hp)] = scores_step(t, hp)
                sc.__exit__(None, None, None)
            att_i[0] += 1
            return True

        for g in range(NG + 1):
            emit_phase_a(g)
            done_g = g - 1  # ropeT for this chunk just emitted
            while att_i[0] < len(steps) + 1:
                i = att_i[0]
                if i < len(steps):
                    t, _hp = steps[i]
                    if 2 * t + 1 > done_g:
                        break
                emit_attention_step()
        while emit_attention_step():
            pass

    nc.compile()
    return nc


def shard_inputs(x, cos, sin, wq, wk, wv, wo):
    """Build per-core input maps: core = b*4 + g. All f16, pre-packed into
    the exact SBUF layouts so every DMA is contiguous per partition."""
    f16 = np.float16
    xts = []
    for b in range(B):
        xb = np.asarray(x[b], dtype=f16).reshape(NG, P, KC, P)
        xts.append(np.ascontiguousarray(xb.transpose(3, 0, 2, 1)).reshape(P, NG * KC * P))
    cs = np.concatenate(
        [
            np.asarray(cos, f16).reshape(NG, P, D).transpose(1, 0, 2).reshape(P, NG * D),
            np.asarray(sin, f16).reshape(NG, P, D).transpose(1, 0, 2).reshape(P, NG * D),
        ],
        axis=1,
    )
    cs = np.ascontiguousarray(cs)
    in_maps = []
    for c in range(N_CORES):
        b, g = divmod(c, N_KV)
        wq_g = np.asarray(wq[:, g * NH * D : (g + 1) * NH * D], f16)
        wq_p = np.ascontiguousarray(
            wq_g.reshape(KC, P, NH * D).transpose(1, 0, 2)
        ).reshape(P, KC * NH * D)
        wk_g = np.asarray(wk[:, g * D : (g + 1) * D], f16).reshape(KC, P, D)
        wv_g = np.asarray(wv[:, g * D : (g + 1) * D], f16).reshape(KC, P, D)
        wkv_p = np.ascontiguousarray(
            np.concatenate([wk_g, wv_g], axis=2).transpose(1, 0, 2)
        ).reshape(P, KC * 2 * D)
        wo_g = np.asarray(wo[g * NH * D : (g + 1) * NH * D, :], f16)
        wo_p = np.ascontiguousarray(
            wo_g.reshape(NH, P, HID).transpose(1, 0, 2)
        ).reshape(P, NH * HID)
        in_maps.append(
            {"xt": xts[b], "cs": cs, "wq": wq_p, "wkv": wkv_p, "wo": wo_p}
        )
    return in_maps


_NC_CACHE = {}


def get_nc():
    if "nc" not in _NC_CACHE:
        _NC_CACHE["nc"] = build_nc()
    return _NC_CACHE["nc"]


def kernel(x, cos, sin, wq, wk, wv, wo, _trace=False):
    from concourse.bass_utils import run_bass_kernel_spmd

    x = np.asarray(x, dtype=np.float32)
    cos = np.asarray(cos, dtype=np.float32)
    sin = np.asarray(sin, dtype=np.float32)
    wq = np.asarray(wq, dtype=np.float32)
    wk = np.asarray(wk, dtype=np.float32)
    wv = np.asarray(wv, dtype=np.float32)
    wo = np.asarray(wo, dtype=np.float32)

    nc = get_nc()
    in_maps = shard_inputs(x, cos, sin, wq, wk, wv, wo)
    res = run_bass_kernel_spmd(nc, in_maps, list(range(N_CORES)), trace=_trace)
    parts = [np.asarray(res.results[c]["out"], dtype=np.float32) for c in range(N_CORES)]
    y = np.stack(
        [sum(parts[b * N_KV + g] for g in range(N_KV)) for b in range(B)], axis=0
    )
    if _trace:
        kernel.last_result = res
    return y


# revision 55
# speedup vs baseline: 1.0390x; 1.0181x over previous
"""Trainium2 Bass kernel for GQA attention with RoPE (B=2, S=1024, HID=2048,
16 q heads / 4 kv heads, head dim 128, causal).

Sharding: 8 cores = 2 batches x 4 kv-head groups. Core c = b*4 + g handles
batch b and kv head g (query heads 4g..4g+3). Each core computes a partial
output y_part = attn_heads @ wo_shard; the host sums the 4 partials per batch.

v12 (final): all-f16 dataflow. Host pre-transposes x and packs every input
into SBUF layout so each DMA is a contiguous per-partition run (no on-device
x transposes; wk/wv previously moved as 256B strided runs, slowing the
fill). Softmax denominator matmuls are halved by pair-summing exp chunks on
the DVE inside each dnpv step, with the PV matmuls emitted first so the PE
covers the add latency. fp8 (incl. DoubleRow) was evaluated and rejected:
HW DoubleRow measures 1.0 cyc/row (2x FLOPs, not the cost model's 4x), and
e4m3's ~6% worst-case element error exceeds the 2e-2 max-error tolerance
through the softmax on every quantization point tested.
"""

import sys

import numpy as np

for _p in ("/opt/trn_rl_repo", "/root/.axon_site/_ro/trn_rl_repo"):
    if _p not in sys.path:
        sys.path.append(_p)

from contextlib import ExitStack

import concourse.bass as bass
import concourse.mybir as mybir
from concourse import bacc
from concourse.masks import make_identity
from concourse.tile import TileContext

P = 128           # partitions / head dim / seq chunk
S = 1024          # sequence length
HID = 2048        # model dim
NH = 4            # query heads per core
D = 128           # head dim
TQ = 256          # query macro-tile
NT = S // TQ      # 4 macro tiles
KC = HID // P     # 16 contraction chunks
NSK = S // P      # 8 key chunks
NG = S // P       # 8 row chunks
F16 = mybir.dt.float16
F32 = mybir.dt.float32
SCALE = 1.0 / float(np.sqrt(D))
AL = mybir.AluOpType
AF = mybir.ActivationFunctionType

N_CORES = 8
B = 2
N_KV = 4
H2 = D // 2


def build_nc(dbg=False):
    nc = bacc.Bacc("TRN2", target_bir_lowering=False, debug=False)
    # all inputs pre-packed by the host into SBUF layout (partition-major,
    # contiguous per-partition runs)
    xt_d = nc.declare_dram_parameter("xt", [P, NG * KC * P], F16, isOutput=False)
    cs_d = nc.declare_dram_parameter("cs", [P, 2 * NG * D], F16, isOutput=False)
    wq_d = nc.declare_dram_parameter("wq", [P, KC * NH * D], F16, isOutput=False)
    wkv_d = nc.declare_dram_parameter("wkv", [P, KC * 2 * D], F16, isOutput=False)
    wo_d = nc.declare_dram_parameter("wo", [P, NH * HID], F16, isOutput=False)
    out_d = nc.declare_dram_parameter("out", [S, HID], F16, isOutput=True)

    with TileContext(nc) as tc, ExitStack() as ctx:
        consts = ctx.enter_context(tc.tile_pool(name="consts", bufs=1))
        wpool = ctx.enter_context(tc.tile_pool(name="wpool", bufs=1))
        persist = ctx.enter_context(tc.tile_pool(name="persist", bufs=1))

        # ---- constants ----
        ident_f32 = consts.tile([P, P], F32, tag="ident_f32")
        make_identity(nc, ident_f32)
        ident = consts.tile([P, P], F16, tag="ident")
        nc.vector.tensor_copy(ident, ident_f32)
        ones = consts.tile([P, P], F16, tag="ones")
        nc.vector.memset(ones, 1.0)
        warm16 = consts.tile([P, 512], F16, tag="warm16")
        nc.vector.memset(warm16, 1.0)
        # causal 0/1 triangle: tri01[k, h, q] = (q >= k), f16, shared by both
        # diagonal chunks of every macro tile
        tri01 = consts.tile([P, 2, P], F16, tag="tri01")
        nc.gpsimd.memset(tri01, 1.0)
        nc.gpsimd.affine_select(
            out=tri01, in_=tri01, compare_op=AL.is_ge, fill=0.0,
            base=0, pattern=[[0, 2], [1, P]], channel_multiplier=-1,
        )

        # ---- persistent weights / tables / activations ----
        # wq in 4 quarter tiles: DMA gating is tile-granular, so quarters
        # let the first q matmuls start after 1MB of DMA instead of 2.5MB
        wq_sbq = [
            wpool.tile([P, 4, NH * D], F16, tag=f"wq{i}", name=f"wq_sbq{i}")
            for i in range(4)
        ]
        wkv_sbA = wpool.tile([P, KC // 2, 2 * D], F16, tag="wkvA")
        wkv_sbB = wpool.tile([P, KC // 2, 2 * D], F16, tag="wkvB")
        wo_sb = wpool.tile([P, NH, HID], F16, tag="wo")
        cos_sb = wpool.tile([P, NG, D], F16, tag="cos")
        sin_sb = wpool.tile([P, NG, D], F16, tag="sin")

        qT_all = persist.tile([P, NH, S], F16, tag="qT")    # [d, h, sq]
        kT = persist.tile([P, S], F16, tag="kT")            # [d, sk]
        vv = persist.tile([P, NSK, D], F16, tag="vv")       # v natural [sk, d]

        # ---- pools ----
        pa = ctx.enter_context(tc.tile_pool(name="pa", bufs=2))
        pb = ctx.enter_context(tc.tile_pool(name="pb", bufs=2))
        ps_mega = ctx.enter_context(tc.tile_pool(name="ps_mega", bufs=7, space="PSUM"))
        ps_qkv = ctx.enter_context(tc.tile_pool(name="ps_qkv", bufs=1, space="PSUM"))

        # warm the PE clock gate while initial DMAs land
        warm_ps = ps_mega.tile([P, 512], F32, tag="mega", name="warm")
        for _ in range(8):
            nc.tensor.matmul(warm_ps[:, 0:P], ones, ones, start=True, stop=True)
        for _ in range(12):
            nc.tensor.matmul(warm_ps, ones, warm16, start=True, stop=True)
        warm_drain = pa.tile([P, 4], F32, tag="warmdrain", bufs=1)
        nc.vector.tensor_copy(warm_drain, warm_ps[:, 0:4])

        # ---- DMAs: xt per-chunk (sync queue), cos/sin on the scalar queue,
        # weights on the sync queue -- v4 order, packed layouts. ----
        x_tiles = [None] * NG

        def emit_xdma(g):
            # two half-tiles per chunk: DMA gating is tile-granular, so the
            # first q matmuls only wait on 0.25MB of x instead of 0.5MB
            xa = pa.tile([P, KC // 2, P], F16, tag="xTa", bufs=4, name=f"xa{g}")
            xb = pa.tile([P, KC // 2, P], F16, tag="xTb", bufs=4, name=f"xb{g}")
            half = KC * P // 2
            nc.sync.dma_start(
                out=xa.rearrange("p c d -> p (c d)"),
                in_=xt_d[:, g * KC * P : g * KC * P + half],
            )
            nc.sync.dma_start(
                out=xb.rearrange("p c d -> p (c d)"),
                in_=xt_d[:, g * KC * P + half : (g + 1) * KC * P],
            )
            x_tiles[g] = (xa, xb)

        def xt_c(g, c):
            return x_tiles[g][c // (KC // 2)][:, c % (KC // 2), :]

        nc.sync.dma_start(
            out=wq_sbq[0].rearrange("p c n -> p (c n)"),
            in_=wq_d[:, 0 : 4 * NH * D],
        )
        emit_xdma(0)
        nc.scalar.dma_start(
            out=cos_sb.rearrange("p g d -> p (g d)"), in_=cs_d[:, 0 : NG * D]
        )
        nc.scalar.dma_start(
            out=sin_sb.rearrange("p g d -> p (g d)"),
            in_=cs_d[:, NG * D : 2 * NG * D],
        )
        for i in range(1, 4):
            nc.sync.dma_start(
                out=wq_sbq[i].rearrange("p c n -> p (c n)"),
                in_=wq_d[:, i * 4 * NH * D : (i + 1) * 4 * NH * D],
            )
        nc.sync.dma_start(
            out=wkv_sbA.rearrange("p c d -> p (c d)"),
            in_=wkv_d[:, 0 : KC * D],
        )
        nc.sync.dma_start(
            out=wkv_sbB.rearrange("p c d -> p (c d)"),
            in_=wkv_d[:, KC * D : KC * 2 * D],
        )
        emit_xdma(1)
        emit_xdma(2)
        emit_xdma(3)
        wo_next = [0]

        def emit_wo_dma():
            h = wo_next[0]
            if h < NH:
                nc.sync.dma_start(
                    out=wo_sb[:, h, :], in_=wo_d[:, h * HID : (h + 1) * HID]
                )
                wo_next[0] += 1

        def bcast_h(ap2d, n):
            """[P, w] slice -> [P, n, w] broadcast AP (0-stride head dim)."""
            return ap2d.rearrange("p (o w) -> p o w", o=1).to_broadcast(
                [P, n, ap2d.shape[-1]]
            )

        # ================= phase A stages =================
        def proj(g):
            """q and kv projections for chunk g (PE, accumulating).
            q uses the dedicated 1-bank pool; kv borrows a mega slot so the
            attention phase gets a 7-deep mega rotation. For g=0 kv goes
            first: it only needs wkv+xt0 (1MB of DMA)."""
            q_ps = ps_qkv.tile([P, NH * D], F32, tag="qkv")
            kv_ps = ps_mega.tile([P, 512], F32, tag="mega", name="kv")[:, 0 : 2 * D]
            for c in range(KC):
                nc.tensor.matmul(
                    q_ps, xt_c(g, c), wq_sbq[c // 4][:, c % 4, :],
                    start=(c == 0), stop=(c == KC - 1),
                )
            for c in range(KC):
                wkv_half = wkv_sbA if c < KC // 2 else wkv_sbB
                nc.tensor.matmul(
                    kv_ps, xt_c(g, c), wkv_half[:, c % (KC // 2), :],
                    start=(c == 0), stop=(c == KC - 1),
                )
            qkv_sb = pa.tile([P, NH * D + 2 * D], F16, tag="qkvsb")
            nc.scalar.activation(out=qkv_sb[:, 0 : NH * D], in_=q_ps, func=AF.Copy)
            nc.scalar.activation(
                out=qkv_sb[:, NH * D : NH * D + 2 * D], in_=kv_ps, func=AF.Copy
            )
            return qkv_sb

        def rope_stage(g, qkv_sb):
            """RoPE on q heads (one 4-head strided pass) + k; v copy-out."""
            q3 = qkv_sb[:, 0 : NH * D].rearrange("p (h d) -> p h d", h=NH)
            k2 = qkv_sb[:, NH * D : NH * D + D]
            cos_g = cos_sb[:, g, :]
            sin_g = sin_sb[:, g, :]

            q_rope = pa.tile([P, NH, D], F16, tag="qrope")
            tmpq = pa.tile([P, NH, D], F16, tag="tmpq")
            nc.vector.scalar_tensor_tensor(
                out=tmpq[:, :, 0:H2], in0=q3[:, :, H2:D], scalar=-1.0,
                in1=bcast_h(sin_g[:, 0:H2], NH), op0=AL.mult, op1=AL.mult,
            )
            nc.vector.tensor_tensor(
                out=tmpq[:, :, H2:D], in0=q3[:, :, 0:H2],
                in1=bcast_h(sin_g[:, H2:D], NH), op=AL.mult,
            )
            nc.vector.tensor_tensor(
                out=q_rope, in0=q3, in1=bcast_h(cos_g, NH), op=AL.mult
            )
            nc.vector.tensor_tensor(out=q_rope, in0=q_rope, in1=tmpq, op=AL.add)

            k_rope = pa.tile([P, D], F16, tag="krope")
            tmpk = pa.tile([P, D], F16, tag="tmpk")
            nc.vector.scalar_tensor_tensor(
                out=tmpk[:, 0:H2], in0=k2[:, H2:D], scalar=-1.0,
                in1=sin_g[:, 0:H2], op0=AL.mult, op1=AL.mult,
            )
            nc.vector.tensor_tensor(
                out=tmpk[:, H2:D], in0=k2[:, 0:H2], in1=sin_g[:, H2:D], op=AL.mult
            )
            nc.vector.tensor_tensor(out=k_rope, in0=k2, in1=cos_g, op=AL.mult)
            nc.vector.tensor_tensor(out=k_rope, in0=k_rope, in1=tmpk, op=AL.add)

            nc.vector.tensor_copy(
                vv[:, g, :], qkv_sb[:, NH * D + D : NH * D + 2 * D]
            )
            return q_rope, k_rope

        def rope_transpose(g, q_rope, k_rope):
            """Transpose RoPE'd q/k into persistent qT_all / kT (PE, f16)."""
            tq_ps = ps_mega.tile([P, 4 * P], F16, tag="mega", name="tq")
            for h in range(NH):
                nc.tensor.transpose(
                    tq_ps[:, h * P : (h + 1) * P], q_rope[:, h, :], ident
                )
            nc.vector.tensor_copy(
                qT_all[:, :, g * P : (g + 1) * P],
                tq_ps.rearrange("p (h d) -> p h d", h=NH),
            )
            tk_ps = ps_mega.tile([P, 4 * P], F16, tag="mega", name="tk")
            nc.tensor.transpose(tk_ps[:, 0:P], k_rope, ident)
            nc.vector.tensor_copy(kT[:, g * P : (g + 1) * P], tk_ps[:, 0:P])

        # ================= phase B stages =================
        def scores_step(t, hp):
            """scoresT + exp for head-pair hp of macro tile t -> expst.

            expst[sk, ik, h2, q]: per key chunk ik, both heads of the pair.
            Diagonal chunks get a post-exp 0/1 triangle multiply; the odd
            diagonal chunk only computes the upper query half."""
            q0 = t * TQ
            nsk = 2 * (t + 1)
            expst = pb.tile([P, NSK, 2, TQ], F16, tag="expst", bufs=2)
            for ik in range(nsk):
                s_ps = ps_mega.tile([P, 2 * TQ], F32, tag="mega", name="s")
                s3 = s_ps.rearrange("p (h q) -> p h q", h=2)
                if ik == nsk - 1:  # odd diagonal: queries q0+128..q0+255 only
                    nc.gpsimd.memset(expst[:, ik, :, 0:P], 0.0)
                    nc.tensor.matmul(
                        s3[:, :, P:TQ],
                        kT[:, ik * P : (ik + 1) * P],
                        qT_all[:, 2 * hp : 2 * hp + 2, q0 + P : q0 + TQ],
                        start=True, stop=True,
                    )
                    nc.scalar.activation(
                        out=expst[:, ik, :, P:TQ], in_=s3[:, :, P:TQ],
                        func=AF.Exp, scale=SCALE,
                    )
                    nc.vector.tensor_tensor(
                        out=expst[:, ik, :, P:TQ], in0=expst[:, ik, :, P:TQ],
                        in1=tri01, op=AL.mult,
                    )
                elif t == NT - 1 and hp == 0:
                    # split into q-chunk halves: the first half only needs
                    # ropeT(2t), so the PE isn't gated on the last ropeT
                    nc.tensor.matmul(
                        s3[:, :, 0:P],
                        kT[:, ik * P : (ik + 1) * P],
                        qT_all[:, 0:2, q0 : q0 + P],
                        start=True, stop=True, skip_group_check=True,
                    )
                    nc.tensor.matmul(
                        s3[:, :, P:TQ],
                        kT[:, ik * P : (ik + 1) * P],
                        qT_all[:, 0:2, q0 + P : q0 + TQ],
                        start=True, stop=True, skip_group_check=True,
                    )
                    nc.scalar.activation(
                        out=expst[:, ik, :, :], in_=s3, func=AF.Exp, scale=SCALE,
                    )
                    if ik == nsk - 2:  # even diagonal: lower-left triangle
                        nc.vector.tensor_tensor(
                            out=expst[:, ik, :, 0:P], in0=expst[:, ik, :, 0:P],
                            in1=tri01, op=AL.mult,
                        )
                else:
                    nc.tensor.matmul(
                        s3,
                        kT[:, ik * P : (ik + 1) * P],
                        qT_all[:, 2 * hp : 2 * hp + 2, q0 : q0 + TQ],
                        start=True, stop=True,
                    )
                    nc.scalar.activation(
                        out=expst[:, ik, :, :], in_=s3, func=AF.Exp, scale=SCALE,
                    )
                    if ik == nsk - 2:  # even diagonal: lower-left triangle
                        nc.vector.tensor_tensor(
                            out=expst[:, ik, :, 0:P], in0=expst[:, ik, :, 0:P],
                            in1=tri01, op=AL.mult,
                        )
            return expst

        def dnpv_step(t, hp, expst, uT_t):
            """DVE pair-sums + PV matmuls + halved denominator matmuls,
            then normalize into uT_t. PV runs while the DVE adds complete."""
            nsk = 2 * (t + 1)
            esum = pb.tile([P, NSK // 2, 2, TQ], F16, tag="esum", bufs=2)
            for j in range(nsk // 2):
                nc.vector.tensor_tensor(
                    out=esum[:, j], in0=expst[:, 2 * j], in1=expst[:, 2 * j + 1],
                    op=AL.add,
                )
            u_ps = ps_mega.tile([P, 2 * TQ], F32, tag="mega", name="u")
            den_ps = ps_mega.tile([P, 2 * TQ], F32, tag="mega", name="den")
            u3 = u_ps.rearrange("p (h q) -> p h q", h=2)
            d3 = den_ps.rearrange("p (h q) -> p h q", h=2)
            for ik in range(nsk):
                last = ik == nsk - 1
                rhs = expst[:, ik, :, P:TQ] if last else expst[:, ik, :, :]
                nc.tensor.matmul(
                    u3[:, :, P:TQ] if last else u3,
                    vv[:, ik, :], rhs,
                    start=(ik == 0), stop=last,
                )
            for j in range(nsk // 2):
                nc.tensor.matmul(
                    d3, ones, esum[:, j],
                    start=(j == 0), stop=(j == nsk // 2 - 1),
                )
            rec = pb.tile([P, 2 * TQ], F32, tag="rec", bufs=2)
            nc.vector.reciprocal_approx_fast(out=rec, in_=den_ps)
            nc.vector.tensor_tensor(
                out=uT_t[:, 2 * hp : 2 * hp + 2, :],
                in0=u3,
                in1=rec.rearrange("p (h q) -> p h q", h=2),
                op=AL.mult,
            )

        def wo_stage(t, uT_t):
            for sub in range(2):
                g = 2 * t + sub
                y_sb = pb.tile([P, HID], F16, tag="ysb", bufs=2)
                for n in range(HID // 512):
                    y_ps = ps_mega.tile([P, 512], F32, tag="mega", name="y")
                    for h in range(NH):
                        nc.tensor.matmul(
                            y_ps,
                            uT_t[:, h, sub * P : (sub + 1) * P],
                            wo_sb[:, h, n * 512 : (n + 1) * 512],
                            start=(h == 0), stop=(h == NH - 1),
                        )
                    if n % 2 == 0:
                        nc.vector.tensor_copy(
                            y_sb[:, n * 512 : (n + 1) * 512], y_ps
                        )
                    else:
                        nc.scalar.activation(
                            out=y_sb[:, n * 512 : (n + 1) * 512], in_=y_ps,
                            func=AF.Copy,
                        )
                    nc.sync.dma_start(
                        out=out_d[g * P : (g + 1) * P, n * 512 : (n + 1) * 512],
                        in_=y_sb[:, n * 512 : (n + 1) * 512],
                    )

        # ================= driver =================
        ropes = [None] * NG
        qkvs = [None] * NG

        def emit_phase_a(g):
            if g in (3, 4):
                emit_wo_dma()
                emit_wo_dma()
            if g >= 1:
                gg = g - 1
                sc = nc.named_scope(f"rope_{gg}"); sc.__enter__()
                ropes[gg] = rope_stage(gg, qkvs[gg])
                sc.__exit__(None, None, None)
                sc = nc.named_scope(f"ropeT_{gg}"); sc.__enter__()
                rope_transpose(gg, *ropes[gg])
                sc.__exit__(None, None, None)
                ropes[gg] = None
                qkvs[gg] = None
            if g < NG:
                if g + 4 < NG:
                    emit_xdma(g + 4)
                sc = nc.named_scope(f"proj_{g}"); sc.__enter__()
                qkvs[g] = proj(g)
                sc.__exit__(None, None, None)

        steps = [(t, hp) for t in range(NT) for hp in range(2)]
        uts = {}
        att_i = [0]

        def emit_attention_step():
            # dnpv/wo (always PE-ready) go before the next scores step,
            # whose matmuls may still be blocked on ropeT of a later chunk.
            i = att_i[0]
            if i >= len(steps) + 1:
                return False
            if 1 <= i:
                t, hp = steps[i - 1]
                sc = nc.named_scope(f"dnpv_{t}_{hp}"); sc.__enter__()
                dnpv_step(t, hp, uts.pop((t, hp)), uts[t])
                sc.__exit__(None, None, None)
                if hp == 1:
                    sc = nc.named_scope(f"wo_{t}"); sc.__enter__()
                    wo_stage(t, uts.pop(t))
                    sc.__exit__(None, None, None)
            if i < len(steps):
                t, hp = steps[i]
                if hp == 0:
                    uts[t] = pb.tile([P, NH, TQ], F16, tag="uT", name=f"uT{t}")
                sc = nc.named_scope(f"sc_{t}_{hp}"); sc.__enter__()
                uts[(t, hp)] = scores_step(t, hp)
                sc.__exit__(None, None, None)
            att_i[0] += 1
            return True

        for g in range(NG + 1):
            emit_phase_a(g)
            done_g = g - 1  # ropeT for this chunk just emitted
            while att_i[0] < len(steps) + 1:
                i = att_i[0]
                if i < len(steps):
                    t, _hp = steps[i]
                    if 2 * t + 1 > done_g:
                        break
                emit_attention_step()
        while emit_attention_step():
            pass

    nc.compile()
    return nc


def shard_inputs(x, cos, sin, wq, wk, wv, wo):
    """Build per-core input maps: core = b*4 + g. All f16, pre-packed into
    the exact SBUF layouts so every DMA is contiguous per partition."""
    f16 = np.float16
    xts = []
    for b in range(B):
        xb = np.asarray(x[b], dtype=f16).reshape(NG, P, KC, P)
        xts.append(np.ascontiguousarray(xb.transpose(3, 0, 2, 1)).reshape(P, NG * KC * P))
    cs = np.concatenate(
        [
            np.asarray(cos, f16).reshape(NG, P, D).transpose(1, 0, 2).reshape(P, NG * D),
            np.asarray(sin, f16).reshape(NG, P, D).transpose(1, 0, 2).reshape(P, NG * D),
        ],
        axis=1,
    )
    cs = np.ascontiguousarray(cs)
    in_maps = []
    for c in range(N_CORES):
        b, g = divmod(c, N_KV)
        wq_g = np.asarray(wq[:, g * NH * D : (g + 1) * NH * D], f16)
        wq_p = np.ascontiguousarray(
            wq_g.reshape(KC, P, NH * D).transpose(1, 0, 2)
        ).reshape(P, KC * NH * D)
        wk_g = np.asarray(wk[:, g * D : (g + 1) * D], f16).reshape(KC, P, D)
        wv_g = np.asarray(wv[:, g * D : (g + 1) * D], f16).reshape(KC, P, D)
        wkv_p = np.ascontiguousarray(
            np.concatenate([wk_g, wv_g], axis=2).transpose(1, 0, 2)
        ).reshape(P, KC * 2 * D)
        wo_g = np.asarray(wo[g * NH * D : (g + 1) * NH * D, :], f16)
        wo_p = np.ascontiguousarray(
            wo_g.reshape(NH, P, HID).transpose(1, 0, 2)
        ).reshape(P, NH * HID)
        in_maps.append(
            {"xt": xts[b], "cs": cs, "wq": wq_p, "wkv": wkv_p, "wo": wo_p}
        )
    return in_maps


_NC_CACHE = {}


def get_nc():
    if "nc" not in _NC_CACHE:
        _NC_CACHE["nc"] = build_nc()
    return _NC_CACHE["nc"]


def kernel(x, cos, sin, wq, wk, wv, wo, _trace=False):
    from concourse.bass_utils import run_bass_kernel_spmd

    x = np.asarray(x, dtype=np.float32)
    cos = np.asarray(cos, dtype=np.float32)
    sin = np.asarray(sin, dtype=np.float32)
    wq = np.asarray(wq, dtype=np.float32)
    wk = np.asarray(wk, dtype=np.float32)
    wv = np.asarray(wv, dtype=np.float32)
    wo = np.asarray(wo, dtype=np.float32)

    nc = get_nc()
    in_maps = shard_inputs(x, cos, sin, wq, wk, wv, wo)
    res = run_bass_kernel_spmd(nc, in_maps, list(range(N_CORES)), trace=_trace)
    parts = [np.asarray(res.results[c]["out"], dtype=np.float32) for c in range(N_CORES)]
    y = np.stack(
        [sum(parts[b * N_KV + g] for g in range(N_KV)) for b in range(B)], axis=0
    )
    if _trace:
        kernel.last_result = res
    return y
